# revision 16
# baseline (speedup 1.0000x reference)
"""Trainium2 Bass kernel for CAM-style channel attention module.

Reference computation (per batch b):
    Q  = W @ X + bias          # 1x1 conv: [256,512]@[512,4096] -> [256,4096]
    E  = Q @ X^T / sqrt(4096)  # [256,512] channel-attention energy
    A  = softmax(E, axis=-1)
    out = gamma * (A @ X) + Q  # residual

Two algebraic tricks remove most of the work:

1. Residual fusion: gamma*(A@X) + (W@X + b) = (W + gamma*A) @ X + b,
   so the residual is one fused bf16 matmul (no Q materialization).

2. Gram route for the energy: E = Q X^T = W (X X^T) + b s^T where
   G = X X^T is the 512x512 Gram matrix and s = row-sums of X.
   G is symmetric, so only the upper block-triangle is computed
   (40960 PE cycles instead of 65536 for Q^T+E per batch); the 6
   lower off-diagonal blocks are cheap 128x128 PE transposes.
   The b s^T term and the bf16-rounding error of G's large diagonal
   (|G_cc| ~ 4096 vs ~64 off-diagonal) are both folded into one
   host-precomputed tile  ebias[q,c'] = b[q] s[c'] + W[q,c'] ddiag[c']
   added to E before the softmax.  Everything stays bf16/fp32 --
   fp8 was measured to break this problem's sharp softmax.

X is uploaded twice (host-prepared): natural [C, HW] bf16 for the
final matmul, and transposed n-tiled [NT, P, C] bf16 for the Gram
matmuls, which removes all on-device X transposes.

Per-core PE budget (2 batches): G 2x41k + mirrors + E 2x4k +
AT 2x1k + final 2x33k cycles ~= 164k cyc ~= 68us warm.
"""

import numpy as np
import ml_dtypes

import concourse.bass as bass
import concourse.tile as tile
from concourse import bacc, mybir
from concourse.bass_utils import run_bass_kernel_spmd

P = 128
NB = 2         # batches per core (B=16 over 8 cores)
C = 512        # input channels
C1 = 256       # conv output channels
HW = 4096      # H*W
CT = C // P    # 4 c-tiles
NT = HW // P   # 32 n-tiles
QH = C1 // P   # 2 q-halves
F32 = mybir.dt.float32
BF16 = mybir.dt.bfloat16
SCALE = 1.0 / 64.0  # 1/sqrt(HW)

XTCH = [2] * 16                  # xtb chunk sizes in n-tiles
XBCH = [512] * 8                 # xb chunk widths in columns
NCHUNK = 512                     # final-matmul free-dim chunk (PSUM bank)

N_CORES = 8


def _bounds(widths):
    b = [0]
    for w in widths:
        b.append(b[-1] + w)
    return b


XTB = _bounds(XTCH)
XBB = _bounds(XBCH)
assert XTB[-1] == NT and XBB[-1] == HW


def build_nc(debug_taps=False):
    nc = bacc.Bacc("TRN2", target_bir_lowering=False, debug=False,
                   num_devices=N_CORES)

    xb_d = nc.dram_tensor("xb_d", [NB, C, HW], BF16, kind="ExternalInput").ap()
    xtb_d = nc.dram_tensor("xtb_d", [NB, NT, P, C], BF16,
                           kind="ExternalInput").ap()
    wt_d = nc.dram_tensor("wt_d", [P, CT, C1], BF16, kind="ExternalInput").ap()
    wtf_d = nc.dram_tensor("wtf_d", [P, CT, C1], F32,
                           kind="ExternalInput").ap()
    eb_d = nc.dram_tensor("eb_d", [NB, P, QH, C], F32,
                          kind="ExternalInput").ap()
    bq_d = nc.dram_tensor("bq_d", [P, QH], F32, kind="ExternalInput").ap()
    gam_d = nc.dram_tensor("gam_d", [P, 1], F32, kind="ExternalInput").ap()
    out = nc.dram_tensor("out", [NB, C1, HW], F32, kind="ExternalOutput").ap()
    if debug_taps:
        dbg_g = nc.dram_tensor("dbg_g", [P, CT, C], BF16,
                               kind="ExternalOutput").ap()
        dbg_a = nc.dram_tensor("dbg_a", [P, QH, C], BF16,
                               kind="ExternalOutput").ap()

    ident_dram = nc.inline_tensor(np.eye(P, dtype=ml_dtypes.bfloat16),
                                  name="ident")

    with tile.TileContext(nc) as tc:
        with (
            tc.tile_pool(name="const", bufs=1) as const,
            tc.tile_pool(name="xs", bufs=2) as xs_pool,
            tc.tile_pool(name="gsb", bufs=2) as g_pool,
            tc.tile_pool(name="sm", bufs=2) as sm_pool,
            tc.tile_pool(name="lhsf", bufs=2) as lhsf_pool,
            tc.tile_pool(name="osb", bufs=3) as osb_pool,
            tc.tile_pool(name="psG", bufs=4, space="PSUM") as psG,
            tc.tile_pool(name="psE", bufs=2, space="PSUM") as psE,
            tc.tile_pool(name="psM", bufs=2, space="PSUM") as psM,
        ):
            # ---- warmup weights: memset, no DMA dependency ----
            warm = const.tile([P, P], BF16)
            nc.vector.memset(warm, 0.0)

            # ---- constants on the scalar HWDGE queue ----
            ident = const.tile([P, P], BF16)
            nc.scalar.dma_start(out=ident, in_=ident_dram.ap())
            wt_sb = const.tile([P, CT, C1], BF16)
            nc.scalar.dma_start(out=wt_sb, in_=wt_d)
            wtf_sb = const.tile([P, CT, C1], F32)
            nc.scalar.dma_start(out=wtf_sb, in_=wtf_d)
            bq_sb = const.tile([P, QH], F32)
            nc.scalar.dma_start(out=bq_sb, in_=bq_d)
            gam_sb = const.tile([P, 1], F32)
            nc.scalar.dma_start(out=gam_sb, in_=gam_d)

            xb_r = xb_d.rearrange("b (ct p) n -> b p ct n", p=P)
            xt_r = xtb_d.rearrange("b nt p c -> b p nt c")
            out_r = out.rearrange("b (t p) n -> b p t n", p=P)

            st = [dict() for _ in range(NB)]

            def issue_xt_dma(bi):
                ch = []
                for j, nn in enumerate(XTCH):
                    t = xs_pool.tile([P, nn, C], BF16, tag=f"xt_{j}",
                                     name=f"xt_{bi}_{j}")
                    eng = nc.sync if j % 2 == 0 else nc.scalar
                    eng.dma_start(out=t,
                                  in_=xt_r[bi][:, XTB[j]:XTB[j + 1]])
                    ch.append(t)
                st[bi]["xt"] = ch

            def issue_eb_dma(bi):
                eb = xs_pool.tile([P, QH, C], F32, tag="eb",
                                  name=f"eb_{bi}")
                nc.scalar.dma_start(out=eb, in_=eb_d[bi])
                st[bi]["eb"] = eb

            def issue_xb_dma(bi, eng):
                ch = []
                for j, w in enumerate(XBCH):
                    t = xs_pool.tile([P, CT, w], BF16, tag=f"xb_{j}",
                                     name=f"xb_{bi}_{j}")
                    eng.dma_start(out=t,
                                  in_=xb_r[bi][:, :, XBB[j]:XBB[j + 1]])
                    ch.append(t)
                st[bi]["xb"] = ch

            def xt_slice(bi, nt):
                """[P, C] bf16 slice for n-tile nt."""
                for j, nn in enumerate(XTCH):
                    if XTB[j] <= nt < XTB[j + 1]:
                        return st[bi]["xt"][j][:, nt - XTB[j]]
                raise AssertionError

            def xb_slice(bi, ct, lo):
                for j, w in enumerate(XBCH):
                    if XBB[j] <= lo < XBB[j + 1]:
                        o = lo - XBB[j]
                        return st[bi]["xb"][j][:, ct, o:o + NCHUNK]
                raise AssertionError

            def emit_G(bi):
                """Upper block-triangle of G = X X^T (4 strips, 32 n-tiles)."""
                ps_g = [psG.tile([P, NCHUNK], F32, tag="g",
                                 name=f"ps_g{bi}{ct}")
                        for ct in range(CT)]
                st[bi]["ps_g"] = ps_g
                for nt in range(NT):
                    xt = xt_slice(bi, nt)
                    for ct in range(CT):
                        w = C - ct * P
                        nc.tensor.matmul(
                            ps_g[ct][:, :w],
                            xt[:, ct * P:(ct + 1) * P],
                            xt[:, ct * P:],
                            start=(nt == 0), stop=(nt == NT - 1))
                # evacuate strips (alternate engines)
                g_sb = g_pool.tile([P, CT, C], BF16, name=f"g_sb{bi}")
                for ct in range(CT):
                    w = C - ct * P
                    if ct % 2 == 0:
                        nc.scalar.copy(out=g_sb[:, ct, ct * P:],
                                       in_=ps_g[ct][:, :w])
                    else:
                        nc.vector.tensor_copy(out=g_sb[:, ct, ct * P:],
                                              in_=ps_g[ct][:, :w])
                st[bi]["g"] = g_sb

            def emit_mirrors(bi):
                """Lower off-diagonal blocks via PE transpose of upper ones."""
                g_sb = st[bi]["g"]
                k = 0
                for ct in range(1, CT):
                    for cs in range(ct):
                        ps_m = psM.tile([P, NCHUNK], F32, tag="m",
                                        name="ps_m")
                        nc.tensor.matmul(
                            ps_m[:, :P],
                            g_sb[:, cs, ct * P:(ct + 1) * P], ident,
                            start=True, stop=True)
                        if k % 2 == 0:
                            nc.scalar.copy(
                                out=g_sb[:, ct, cs * P:(cs + 1) * P],
                                in_=ps_m[:, :P])
                        else:
                            nc.vector.tensor_copy(
                                out=g_sb[:, ct, cs * P:(cs + 1) * P],
                                in_=ps_m[:, :P])
                        k += 1

            def emit_E(bi):
                """E = W @ G accumulated over the 4 c-strips."""
                g_sb = st[bi]["g"]
                ps_e = [psE.tile([P, C], F32, tag="e", name=f"ps_e{bi}{qh}")
                        for qh in range(QH)]
                st[bi]["ps_e"] = ps_e
                for ct in range(CT):
                    for qh in range(QH):
                        nc.tensor.matmul(
                            ps_e[qh],
                            wt_sb[:, ct, qh * P:(qh + 1) * P],
                            g_sb[:, ct, :],
                            start=(ct == 0), stop=(ct == CT - 1))

            def emit_softmax(bi):
                a_scaled = sm_pool.tile([P, QH, C], BF16, tag="a",
                                        name=f"a_scaled{bi}")
                for qh in range(QH):
                    e_sb = sm_pool.tile([P, C], F32, tag="esb")
                    nc.vector.tensor_add(out=e_sb, in0=st[bi]["ps_e"][qh],
                                         in1=st[bi]["eb"][:, qh, :])
                    mx = sm_pool.tile([P, 1], F32, tag="mx")
                    nc.vector.reduce_max(mx, e_sb,
                                         axis=mybir.AxisListType.X,
                                         negate=True)
                    nbias = sm_pool.tile([P, 1], F32, tag="nb")
                    nc.vector.tensor_scalar_mul(nbias, mx, SCALE)
                    a_f = sm_pool.tile([P, C], F32, tag="af")
                    rs = sm_pool.tile([P, 1], F32, tag="rs")
                    nc.scalar.activation(
                        out=a_f, in_=e_sb,
                        func=mybir.ActivationFunctionType.Exp,
                        bias=nbias, scale=SCALE, accum_out=rs)
                    rc = sm_pool.tile([P, 1], F32, tag="rc")
                    nc.vector.reciprocal(rc, rs)
                    sc = sm_pool.tile([P, 1], F32, tag="sc")
                    nc.vector.tensor_mul(sc, rc, gam_sb)
                    nc.vector.tensor_scalar_mul(a_scaled[:, qh, :], a_f, sc)
                st[bi]["a"] = a_scaled

            def emit_ATcombine(bi):
                lhsf = lhsf_pool.tile([P, CT, C1], BF16, name=f"lhsf{bi}")
                a_scaled = st[bi]["a"]
                for ct in range(CT):
                    ps_at = psM.tile([P, NCHUNK], F32, tag="m", name="ps_at")
                    for qh in range(QH):
                        nc.tensor.matmul(
                            ps_at[:, qh * P:(qh + 1) * P],
                            a_scaled[:, qh, ct * P:(ct + 1) * P], ident,
                            start=True, stop=True)
                    nc.vector.tensor_add(
                        out=lhsf[:, ct, :], in0=ps_at[:, :C1],
                        in1=wtf_sb[:, ct, :])
                st[bi]["lhsf"] = lhsf

            def emit_F_group(bi, qh, ng):
                """final = (W + gamma*A)^T.T @ X + b for 4 n-chunks."""
                lhsf = st[bi]["lhsf"]
                o_sb = osb_pool.tile([P, 4 * NCHUNK], F32, tag="o")
                for sub in range(4):
                    lo = (ng * 4 + sub) * NCHUNK
                    ps_o = psM.tile([P, NCHUNK], F32, tag="m", name="ps_o")
                    for ct in range(CT):
                        nc.tensor.matmul(
                            ps_o, lhsf[:, ct, qh * P:(qh + 1) * P],
                            xb_slice(bi, ct, lo),
                            start=(ct == 0), stop=(ct == CT - 1))
                    oslice = o_sb[:, sub * NCHUNK:(sub + 1) * NCHUNK]
                    if sub % 2 == 0:
                        nc.scalar.add(out=oslice, in_=ps_o,
                                      add=bq_sb[:, qh:qh + 1])
                    else:
                        nc.vector.tensor_scalar_add(oslice, ps_o,
                                                    bq_sb[:, qh:qh + 1])
                nc.sync.dma_start(
                    out=out_r[bi, :, qh,
                              ng * 4 * NCHUNK:(ng * 4 + 4) * NCHUNK],
                    in_=o_sb)

            # ---- DMA issue (program order per queue == transfer order) ----
            issue_xt_dma(0)
            issue_eb_dma(0)
            issue_xt_dma(1)
            issue_eb_dma(1)
            issue_xb_dma(0, nc.gpsimd)
            issue_xb_dma(1, nc.gpsimd)

            # ---- HAM warm-up on the memset tile while DMAs are in flight
            ps_w = psM.tile([P, NCHUNK], F32, tag="m", name="warm_ps")
            NWARM = 44
            for wj in range(NWARM):
                nc.tensor.matmul(ps_w[:, :P], warm, warm,
                                 start=(wj == 0), stop=(wj == NWARM - 1))

            # ---- the schedule ----
            emit_G(0)
            emit_mirrors(0)
            emit_E(0)
            if debug_taps:
                nc.sync.dma_start(out=dbg_g, in_=st[0]["g"])
            emit_softmax(0)
            if debug_taps:
                nc.sync.dma_start(out=dbg_a, in_=st[0]["a"])
            emit_G(1)
            emit_mirrors(1)
            emit_E(1)
            emit_ATcombine(0)
            emit_F_group(0, 0, 0)
            emit_softmax(1)
            emit_F_group(0, 0, 1)
            emit_F_group(0, 1, 0)
            emit_F_group(0, 1, 1)
            emit_ATcombine(1)
            for qh in range(QH):
                for ng in range(2):
                    emit_F_group(1, qh, ng)
    nc.compile()
    return nc


_NC_CACHE = None


def _get_nc():
    global _NC_CACHE
    if _NC_CACHE is None:
        _NC_CACHE = build_nc()
    return _NC_CACHE


def make_in_maps(x, conv_w, conv_b, gamma):
    B = x.shape[0]
    xs = np.ascontiguousarray(x.reshape(B, C, HW), dtype=np.float32)
    xb_np = xs.astype(ml_dtypes.bfloat16)                      # [B, C, HW]
    xtb_np = np.ascontiguousarray(
        xb_np.transpose(0, 2, 1)).reshape(B, NT, P, C)         # [B,NT,P,C]
    wm = conv_w.reshape(C1, C).astype(np.float32)
    wt_tiled = np.ascontiguousarray(
        wm.T.reshape(CT, P, C1).transpose(1, 0, 2))            # [P, CT, C1]
    wt16 = wt_tiled.astype(ml_dtypes.bfloat16)
    b_np = conv_b.astype(np.float32)
    # ebias[b, q, c] = b[q] * s[b, c] + W[q, c] * ddiag[b, c]
    x16f = xb_np.astype(np.float32)
    s = x16f.sum(axis=2, dtype=np.float64).astype(np.float32)  # [B, C]
    diag = (x16f * x16f).sum(axis=2, dtype=np.float64).astype(np.float32)
    ddiag = diag - diag.astype(ml_dtypes.bfloat16).astype(np.float32)
    ebias = (b_np[None, :, None] * s[:, None, :]
             + wm[None, :, :] * ddiag[:, None, :])             # [B, C1, C]
    ebias = np.ascontiguousarray(
        ebias.reshape(B, QH, P, C).transpose(0, 2, 1, 3))      # [B, P, QH, C]
    bq = np.ascontiguousarray(b_np.reshape(QH, P).T)           # [P, QH]
    gam = np.ascontiguousarray(
        np.broadcast_to(gamma.astype(np.float32).reshape(1, 1), (P, 1)))
    in_maps = []
    for ci in range(N_CORES):
        sl = slice(NB * ci, NB * (ci + 1))
        in_maps.append({
            "xb_d": np.ascontiguousarray(xb_np[sl]),
            "xtb_d": np.ascontiguousarray(xtb_np[sl]),
            "wt_d": wt16,
            "wtf_d": wt_tiled,
            "eb_d": np.ascontiguousarray(ebias[sl]),
            "bq_d": bq,
            "gam_d": gam,
        })
    return in_maps


def kernel(x, conv_w, conv_b, gamma, trace=False):
    """Full inputs in, full output out. Shards batch over 8 NeuronCores."""
    nc = _get_nc()
    in_maps = make_in_maps(x, conv_w, conv_b, gamma)
    res = run_bass_kernel_spmd(nc, in_maps, core_ids=list(range(N_CORES)),
                               trace=trace)
    outs = [r["out"].reshape(NB, C1, 64, 64) for r in res.results]
    full = np.concatenate(outs, axis=0).astype(np.float32)
    if trace:
        kernel.last_results = res
    return full


kernel.last_results = None


# revision 22
# speedup vs baseline: 1.1592x; 1.1592x over previous
"""Trainium2 Bass kernel for CAM-style channel attention module.

Reference computation (per batch b):
    Q  = W @ X + bias          # 1x1 conv: [256,512]@[512,4096] -> [256,4096]
    E  = Q @ X^T / sqrt(4096)  # [256,512] channel-attention energy
    A  = softmax(E, axis=-1)
    out = gamma * (A @ X) + Q  # residual

Two algebraic tricks remove most of the work:

1. Residual fusion: gamma*(A@X) + (W@X + b) = (W + gamma*A) @ X + b,
   so the residual is one fused bf16 matmul (no Q materialization).

2. Gram route for the energy: E = Q X^T = W (X X^T) + b s^T where
   G = X X^T is the 512x512 Gram matrix and s = row-sums of X.
   G is symmetric, so only the upper block-triangle is computed
   (40960 PE cycles instead of 65536 for Q^T+E per batch); the 6
   lower off-diagonal blocks are cheap 128x128 PE transposes.
   The b s^T term and the bf16-rounding error of G's large diagonal
   (|G_cc| ~ 4096 vs ~64 off-diagonal) are both folded into one
   host-precomputed tile  ebias[q,c'] = b[q] s[c'] + W[q,c'] ddiag[c']
   added to E before the softmax.  Everything stays bf16/fp32 --
   fp8 was measured to break this problem's sharp softmax.

X is uploaded twice (host-prepared): natural [C, HW] bf16 for the
final matmul, and transposed n-tiled [NT, P, C] bf16 for the Gram
matmuls, which removes all on-device X transposes.

Per-core PE budget (2 batches): G 2x41k + mirrors + E 2x4k +
AT 2x1k + final 2x33k cycles ~= 164k cyc ~= 68us warm.
"""

import numpy as np
import ml_dtypes

import concourse.bass as bass
import concourse.tile as tile
from concourse import bacc, mybir
from concourse.bass_utils import run_bass_kernel_spmd

P = 128
NB = 2         # batches per core (B=16 over 8 cores)
C = 512        # input channels
C1 = 256       # conv output channels
HW = 4096      # H*W
CT = C // P    # 4 c-tiles
NT = HW // P   # 32 n-tiles
QH = C1 // P   # 2 q-halves
F32 = mybir.dt.float32
BF16 = mybir.dt.bfloat16
SCALE = 1.0 / 64.0  # 1/sqrt(HW)

XTCH = [2] * 16                  # xtb chunk sizes in n-tiles
XBCH = [2048, 2048]              # xb chunk widths in columns
NCHUNK = 512                     # final-matmul free-dim chunk (PSUM bank)

N_CORES = 8


def _bounds(widths):
    b = [0]
    for w in widths:
        b.append(b[-1] + w)
    return b


XTB = _bounds(XTCH)
XBB = _bounds(XBCH)
assert XTB[-1] == NT and XBB[-1] == HW


def build_nc(debug_taps=False):
    nc = bacc.Bacc("TRN2", target_bir_lowering=False, debug=False,
                   num_devices=N_CORES)

    xb_d = nc.dram_tensor("xb_d", [NB, C, HW], BF16, kind="ExternalInput").ap()
    xtb_d = nc.dram_tensor("xtb_d", [NB, P, NT * C], BF16,
                           kind="ExternalInput").ap()
    wt_d = nc.dram_tensor("wt_d", [P, CT, C1], BF16, kind="ExternalInput").ap()
    wtf_d = nc.dram_tensor("wtf_d", [P, CT, C1], F32,
                           kind="ExternalInput").ap()
    eb_d = nc.dram_tensor("eb_d", [NB, P, QH, C], F32,
                          kind="ExternalInput").ap()
    bq_d = nc.dram_tensor("bq_d", [P, QH], F32, kind="ExternalInput").ap()
    gam_d = nc.dram_tensor("gam_d", [P, 1], F32, kind="ExternalInput").ap()
    out = nc.dram_tensor("out", [NB, C1, HW], F32, kind="ExternalOutput").ap()
    if debug_taps:
        dbg_g = nc.dram_tensor("dbg_g", [P, CT, C], BF16,
                               kind="ExternalOutput").ap()
        dbg_a = nc.dram_tensor("dbg_a", [P, QH, C], BF16,
                               kind="ExternalOutput").ap()

    ident_dram = nc.inline_tensor(np.eye(P, dtype=ml_dtypes.bfloat16),
                                  name="ident")

    with tile.TileContext(nc) as tc:
        with (
            tc.tile_pool(name="const", bufs=1) as const,
            tc.tile_pool(name="xs", bufs=2) as xs_pool,
            tc.tile_pool(name="gsb", bufs=2) as g_pool,
            tc.tile_pool(name="sm", bufs=2) as sm_pool,
            tc.tile_pool(name="lhsf", bufs=2) as lhsf_pool,
            tc.tile_pool(name="osb", bufs=3) as osb_pool,
            tc.tile_pool(name="psG", bufs=4, space="PSUM") as psG,
            tc.tile_pool(name="psE", bufs=2, space="PSUM") as psE,
            tc.tile_pool(name="psM", bufs=2, space="PSUM") as psM,
        ):
            # ---- warmup weights: memset, no DMA dependency ----
            warm = const.tile([P, P], BF16)
            nc.vector.memset(warm, 0.0)

            # ---- constants on the scalar HWDGE queue ----
            ident = const.tile([P, P], BF16)
            nc.scalar.dma_start(out=ident, in_=ident_dram.ap())
            wt_sb = const.tile([P, CT, C1], BF16)
            nc.scalar.dma_start(out=wt_sb, in_=wt_d)
            wtf_sb = const.tile([P, CT, C1], F32)
            nc.scalar.dma_start(out=wtf_sb, in_=wtf_d)
            bq_sb = const.tile([P, QH], F32)
            nc.scalar.dma_start(out=bq_sb, in_=bq_d)
            gam_sb = const.tile([P, 1], F32)
            nc.scalar.dma_start(out=gam_sb, in_=gam_d)

            xb_r = xb_d.rearrange("b (ct p) n -> b p ct n", p=P)
            out_r = out.rearrange("b (t p) n -> b p t n", p=P)

            st = [dict() for _ in range(NB)]

            ENGS3 = None

            def issue_xt_dma(bi):
                ch = []
                for j, nn in enumerate(XTCH):
                    t = xs_pool.tile([P, nn, C], BF16, tag=f"xt_{j}",
                                     name=f"xt_{bi}_{j}")
                    eng = ENGS3[j % 3]
                    eng.dma_start(
                        out=t,
                        in_=xtb_d[bi][:, XTB[j] * C:XTB[j + 1] * C])
                    ch.append(t)
                st[bi]["xt"] = ch

            def issue_eb_dma(bi):
                eb = xs_pool.tile([P, QH, C], F32, tag="eb",
                                  name=f"eb_{bi}")
                nc.gpsimd.dma_start(out=eb, in_=eb_d[bi])
                st[bi]["eb"] = eb

            def issue_xb_dma(bi, eng):
                ch = []
                for j, w in enumerate(XBCH):
                    t = xs_pool.tile([P, CT, w], BF16, tag=f"xb_{j}",
                                     name=f"xb_{bi}_{j}")
                    eng.dma_start(out=t,
                                  in_=xb_r[bi][:, :, XBB[j]:XBB[j + 1]])
                    ch.append(t)
                st[bi]["xb"] = ch

            def xt_slice(bi, nt):
                """[P, C] bf16 slice for n-tile nt."""
                for j, nn in enumerate(XTCH):
                    if XTB[j] <= nt < XTB[j + 1]:
                        return st[bi]["xt"][j][:, nt - XTB[j]]
                raise AssertionError

            def xb_slice(bi, ct, lo):
                for j, w in enumerate(XBCH):
                    if XBB[j] <= lo < XBB[j + 1]:
                        o = lo - XBB[j]
                        return st[bi]["xb"][j][:, ct, o:o + NCHUNK]
                raise AssertionError

            def emit_G(bi):
                """Upper block-triangle of G = X X^T (4 strips, 32 n-tiles)."""
                ps_g = [psG.tile([P, NCHUNK], F32, tag="g",
                                 name=f"ps_g{bi}{ct}")
                        for ct in range(CT)]
                st[bi]["ps_g"] = ps_g
                for nt in range(NT):
                    xt = xt_slice(bi, nt)
                    for ct in range(CT):
                        w = C - ct * P
                        nc.tensor.matmul(
                            ps_g[ct][:, :w],
                            xt[:, ct * P:(ct + 1) * P],
                            xt[:, ct * P:],
                            start=(nt == 0), stop=(nt == NT - 1))
                # evacuate strips (alternate engines)
                g_sb = g_pool.tile([P, CT, C], BF16, name=f"g_sb{bi}")
                for ct in range(CT):
                    w = C - ct * P
                    if ct % 2 == 0:
                        nc.scalar.copy(out=g_sb[:, ct, ct * P:],
                                       in_=ps_g[ct][:, :w])
                    else:
                        nc.vector.tensor_copy(out=g_sb[:, ct, ct * P:],
                                              in_=ps_g[ct][:, :w])
                st[bi]["g"] = g_sb

            def emit_mirrors(bi):
                """Lower off-diagonal blocks via PE transpose of upper ones."""
                g_sb = st[bi]["g"]
                k = 0
                for ct in range(1, CT):
                    for cs in range(ct):
                        ps_m = psM.tile([P, NCHUNK], F32, tag="m",
                                        name="ps_m")
                        nc.tensor.matmul(
                            ps_m[:, :P],
                            g_sb[:, cs, ct * P:(ct + 1) * P], ident,
                            start=True, stop=True)
                        if k % 2 == 0:
                            nc.scalar.copy(
                                out=g_sb[:, ct, cs * P:(cs + 1) * P],
                                in_=ps_m[:, :P])
                        else:
                            nc.vector.tensor_copy(
                                out=g_sb[:, ct, cs * P:(cs + 1) * P],
                                in_=ps_m[:, :P])
                        k += 1

            def emit_E(bi):
                """E = W @ G accumulated over the 4 c-strips."""
                g_sb = st[bi]["g"]
                ps_e = [psE.tile([P, C], F32, tag="e", name=f"ps_e{bi}{qh}")
                        for qh in range(QH)]
                st[bi]["ps_e"] = ps_e
                for ct in range(CT):
                    for qh in range(QH):
                        nc.tensor.matmul(
                            ps_e[qh],
                            wt_sb[:, ct, qh * P:(qh + 1) * P],
                            g_sb[:, ct, :],
                            start=(ct == 0), stop=(ct == CT - 1))

            def emit_softmax(bi):
                a_scaled = sm_pool.tile([P, QH, C], BF16, tag="a",
                                        name=f"a_scaled{bi}")
                for qh in range(QH):
                    e_sb = sm_pool.tile([P, C], F32, tag="esb")
                    nc.vector.tensor_add(out=e_sb, in0=st[bi]["ps_e"][qh],
                                         in1=st[bi]["eb"][:, qh, :])
                    mx = sm_pool.tile([P, 1], F32, tag="mx")
                    nc.vector.reduce_max(mx, e_sb,
                                         axis=mybir.AxisListType.X,
                                         negate=True)
                    nbias = sm_pool.tile([P, 1], F32, tag="nb")
                    nc.vector.tensor_scalar_mul(nbias, mx, SCALE)
                    a_f = sm_pool.tile([P, C], F32, tag="af")
                    rs = sm_pool.tile([P, 1], F32, tag="rs")
                    nc.scalar.activation(
                        out=a_f, in_=e_sb,
                        func=mybir.ActivationFunctionType.Exp,
                        bias=nbias, scale=SCALE, accum_out=rs)
                    rc = sm_pool.tile([P, 1], F32, tag="rc")
                    nc.vector.reciprocal(rc, rs)
                    sc = sm_pool.tile([P, 1], F32, tag="sc")
                    nc.vector.tensor_mul(sc, rc, gam_sb)
                    nc.vector.tensor_scalar_mul(a_scaled[:, qh, :], a_f, sc)
                st[bi]["a"] = a_scaled

            def emit_ATcombine(bi):
                lhsf = lhsf_pool.tile([P, CT, C1], BF16, name=f"lhsf{bi}")
                a_scaled = st[bi]["a"]
                for ct in range(CT):
                    ps_at = psM.tile([P, NCHUNK], F32, tag="m", name="ps_at")
                    for qh in range(QH):
                        nc.tensor.matmul(
                            ps_at[:, qh * P:(qh + 1) * P],
                            a_scaled[:, qh, ct * P:(ct + 1) * P], ident,
                            start=True, stop=True)
                    nc.vector.tensor_add(
                        out=lhsf[:, ct, :], in0=ps_at[:, :C1],
                        in1=wtf_sb[:, ct, :])
                st[bi]["lhsf"] = lhsf

            def emit_F_group(bi, qh, ng):
                """final = (W + gamma*A)^T.T @ X + b for 4 n-chunks."""
                lhsf = st[bi]["lhsf"]
                o_sb = osb_pool.tile([P, 4 * NCHUNK], F32, tag="o")
                for sub in range(4):
                    lo = (ng * 4 + sub) * NCHUNK
                    ps_o = psM.tile([P, NCHUNK], F32, tag="m", name="ps_o")
                    for ct in range(CT):
                        nc.tensor.matmul(
                            ps_o, lhsf[:, ct, qh * P:(qh + 1) * P],
                            xb_slice(bi, ct, lo),
                            start=(ct == 0), stop=(ct == CT - 1))
                    oslice = o_sb[:, sub * NCHUNK:(sub + 1) * NCHUNK]
                    if sub % 2 == 0:
                        nc.scalar.add(out=oslice, in_=ps_o,
                                      add=bq_sb[:, qh:qh + 1])
                    else:
                        nc.vector.tensor_scalar_add(oslice, ps_o,
                                                    bq_sb[:, qh:qh + 1])
                nc.sync.dma_start(
                    out=out_r[bi, :, qh,
                              ng * 4 * NCHUNK:(ng * 4 + 4) * NCHUNK],
                    in_=o_sb)

            # ---- DMA issue (program order per queue == transfer order) ----
            ENGS3 = [nc.sync, nc.scalar, nc.gpsimd]
            issue_eb_dma(0)
            issue_eb_dma(1)
            issue_xt_dma(0)
            issue_xt_dma(1)
            issue_xb_dma(0, nc.sync)
            issue_xb_dma(1, nc.scalar)

            # ---- HAM warm-up on the memset tile while DMAs are in flight
            ps_w = psM.tile([P, NCHUNK], F32, tag="m", name="warm_ps")
            NWARM = 44
            for wj in range(NWARM):
                nc.tensor.matmul(ps_w[:, :P], warm, warm,
                                 start=(wj == 0), stop=(wj == NWARM - 1))

            # ---- the schedule ----
            emit_G(0)
            emit_mirrors(0)
            emit_E(0)
            if debug_taps:
                nc.sync.dma_start(out=dbg_g, in_=st[0]["g"])
            emit_softmax(0)
            if debug_taps:
                nc.sync.dma_start(out=dbg_a, in_=st[0]["a"])
            emit_G(1)
            emit_mirrors(1)
            emit_E(1)
            emit_ATcombine(0)
            emit_F_group(0, 0, 0)
            emit_softmax(1)
            emit_F_group(0, 0, 1)
            emit_F_group(0, 1, 0)
            emit_F_group(0, 1, 1)
            emit_ATcombine(1)
            for qh in range(QH):
                for ng in range(2):
                    emit_F_group(1, qh, ng)
    nc.compile()
    return nc


_NC_CACHE = None


def _get_nc():
    global _NC_CACHE
    if _NC_CACHE is None:
        _NC_CACHE = build_nc()
    return _NC_CACHE


def make_in_maps(x, conv_w, conv_b, gamma):
    B = x.shape[0]
    xs = np.ascontiguousarray(x.reshape(B, C, HW), dtype=np.float32)
    xb_np = xs.astype(ml_dtypes.bfloat16)                      # [B, C, HW]
    xtb_np = np.ascontiguousarray(
        xb_np.transpose(0, 2, 1).reshape(B, NT, P, C)
        .transpose(0, 2, 1, 3)).reshape(B, P, NT * C)          # [B,P,NT*C]
    wm = conv_w.reshape(C1, C).astype(np.float32)
    wt_tiled = np.ascontiguousarray(
        wm.T.reshape(CT, P, C1).transpose(1, 0, 2))            # [P, CT, C1]
    wt16 = wt_tiled.astype(ml_dtypes.bfloat16)
    b_np = conv_b.astype(np.float32)
    # ebias[b, q, c] = b[q] * s[b, c] + W[q, c] * ddiag[b, c]
    x16f = xb_np.astype(np.float32)
    s = x16f.sum(axis=2, dtype=np.float64).astype(np.float32)  # [B, C]
    diag = (x16f * x16f).sum(axis=2, dtype=np.float64).astype(np.float32)
    ddiag = diag - diag.astype(ml_dtypes.bfloat16).astype(np.float32)
    ebias = (b_np[None, :, None] * s[:, None, :]
             + wm[None, :, :] * ddiag[:, None, :])             # [B, C1, C]
    ebias = np.ascontiguousarray(
        ebias.reshape(B, QH, P, C).transpose(0, 2, 1, 3))      # [B, P, QH, C]
    bq = np.ascontiguousarray(b_np.reshape(QH, P).T)           # [P, QH]
    gam = np.ascontiguousarray(
        np.broadcast_to(gamma.astype(np.float32).reshape(1, 1), (P, 1)))
    in_maps = []
    for ci in range(N_CORES):
        sl = slice(NB * ci, NB * (ci + 1))
        in_maps.append({
            "xb_d": np.ascontiguousarray(xb_np[sl]),
            "xtb_d": np.ascontiguousarray(xtb_np[sl]),
            "wt_d": wt16,
            "wtf_d": wt_tiled,
            "eb_d": np.ascontiguousarray(ebias[sl]),
            "bq_d": bq,
            "gam_d": gam,
        })
    return in_maps


def kernel(x, conv_w, conv_b, gamma, trace=False):
    """Full inputs in, full output out. Shards batch over 8 NeuronCores."""
    nc = _get_nc()
    in_maps = make_in_maps(x, conv_w, conv_b, gamma)
    res = run_bass_kernel_spmd(nc, in_maps, core_ids=list(range(N_CORES)),
                               trace=trace)
    outs = [r["out"].reshape(NB, C1, 64, 64) for r in res.results]
    full = np.concatenate(outs, axis=0).astype(np.float32)
    if trace:
        kernel.last_results = res
    return full


kernel.last_results = None
